# revision 1
# baseline (speedup 1.0000x reference)
"""AttentiveStatsPooling Trainium2 kernel.

Full-input contract: kernel(**inputs) takes the unsharded numpy inputs
  x            (32, 1536, 2048) f32
  padding_mask (32, 2048)       bool
  W_tdnn       (128, 1536)      f32
  b_tdnn       (128,)           f32
  W_attn       (1536, 128)      f32
  b_attn       (1536,)          f32
and returns the full (32, 3072) f32 output.

Sharding: data-parallel over batch. 8 cores x 4 samples each, weights
replicated. Math per sample:
  e    = tanh(W_tdnn @ x + b_tdnn)            (BN, T)
  a    = W_attn @ e  (+ b_attn: dropped - constant along T, cancels in
                      the softmax over T)      (C, T)
  a   += -1e9 * mask[t]                        (additive mask; exp -> 0)
  S0   = sum_t exp(a);  S1 = sum_t exp(a)*x;  S2 = sum_t exp(a)*x^2
  mean = S1/S0;  std = sqrt(clip(S2/S0 - mean^2, 1e-9))
All matmuls/products in bf16 with fp32 accumulation (PSUM / reduce
accumulators): HW-verified relative error 2.1e-4 (scale-rel absmax 7.5e-4).

Performance (measured on HW, ~300-330 us/core; 1.7x over the naive
schedule). Engine assignment chosen from on-HW microbenchmarks:
  - any DVE op with accum_out runs 1x (fast perf-modes disabled by the
    accumulator), so reductions cost ~2.2us/[128,2048] everywhere;
  - products (tensor_tensor bf16) do hit the 2x mode (1.17us);
  - exp on ACT reads logits straight from PSUM, its accumulator gives
    S0 for free; S1 reduces on DVE, S2 on ACT (Copy+accum) to balance
    both engines at ~235us busy;
  - the S2 stage is pipelined 2 steps behind, and two samples' chunk
    streams are interleaved so each engine fills the other stream's
    cross-engine dependency bubbles (the single biggest win).
"""

import numpy as np
import ml_dtypes

B, C, T = 32, 1536, 2048
BN = 128
NCORES = 8
SPC = B // NCORES  # samples per core
CK = C // 128      # c chunks of 128 partitions
NJ = T // 512      # 512-wide column groups (one PSUM bank each)

BF16 = ml_dtypes.bfloat16

_PROG_CACHE = {}


def _build_program(reps=None):
    """Build the per-core program. reps=None: straight-line body.
    reps=K: wrap the whole body in a hardware For_i loop (timing only)."""
    import concourse.bacc as bacc
    import concourse.tile as tile
    import concourse.mybir as mybir
    from contextlib import nullcontext
    from concourse.bass_interp import get_hw_module

    dt = mybir.dt
    AF = mybir.ActivationFunctionType
    OP = mybir.AluOpType

    nc = bacc.Bacc(
        "TRN2",
        target_bir_lowering=False,
        debug=False,
        num_devices=NCORES,
        num_swdge_queues=4,
    )
    x_d = nc.dram_tensor("x", [SPC, C, T], dt.bfloat16, kind="ExternalInput")
    mn_d = nc.dram_tensor("maskneg", [SPC, T], dt.bfloat16, kind="ExternalInput")
    wt_d = nc.dram_tensor("wt", [C, BN], dt.bfloat16, kind="ExternalInput")
    wa_d = nc.dram_tensor("wa", [BN, C], dt.bfloat16, kind="ExternalInput")
    bt_d = nc.dram_tensor("bt", [BN, 1], dt.float32, kind="ExternalInput")
    out_d = nc.dram_tensor("out", [SPC, 2 * C], dt.float32, kind="ExternalOutput")

    with tile.TileContext(nc) as tc:
        with (
            tc.tile_pool(name="const", bufs=1) as constp,
            tc.tile_pool(name="xin", bufs=2 * CK) as xp,
            tc.tile_pool(name="esb", bufs=3) as ep,
            tc.tile_pool(name="expm", bufs=3) as xpm,
            tc.tile_pool(name="prod", bufs=4) as prp,
            tc.tile_pool(name="mneg", bufs=2) as mnp,
            tc.tile_pool(name="s0p", bufs=4) as s0pp,
            tc.tile_pool(name="stats", bufs=1) as statsp,
            tc.tile_pool(name="tail", bufs=2) as tailp,
            tc.tile_pool(name="ps", bufs=2, space="PSUM") as psp,
        ):
            # ---- constants ------------------------------------------------
            wt_sb = constp.tile([128, CK, BN], dt.bfloat16, tag="wt")
            nc.sync.dma_start(
                out=wt_sb, in_=wt_d.ap().rearrange("(k p) o -> p k o", p=128)
            )
            wa_sb = constp.tile([128, C], dt.bfloat16, tag="wa")
            nc.sync.dma_start(out=wa_sb, in_=wa_d.ap())
            bt_sb = constp.tile([128, 1], dt.float32, tag="bt")
            nc.sync.dma_start(out=bt_sb, in_=bt_d.ap())
            ones_sb = constp.tile([1, 128], dt.bfloat16, tag="ones")
            nc.vector.memset(ones_sb, 1.0)

            loop_cm = tc.For_i(0, reps, 1) if reps is not None else nullcontext()
            with loop_cm:
                stats = []
                for s in range(SPC):
                    S0 = statsp.tile([128, CK], dt.float32, tag=f"S0_{s}")
                    S1 = statsp.tile([128, CK], dt.float32, tag=f"S1_{s}")
                    S2 = statsp.tile([128, CK], dt.float32, tag=f"S2_{s}")
                    stats.append((S0, S1, S2))

                # process samples in pairs; the two chunk streams interleave
                # so ACT/DVE always have an independent chunk to work on
                for s0 in range(0, SPC, 2):
                    pair = [s0, s0 + 1]
                    xts = {}
                    mnegs = {}
                    esbs = {}
                    for s in pair:
                        mneg_sb = mnp.tile(
                            [1, T], dt.bfloat16, tag="mneg", name=f"mneg_{s}"
                        )
                        nc.sync.dma_start(out=mneg_sb, in_=mn_d.ap()[s : s + 1, :])
                        mnegs[s] = mneg_sb
                        for k in range(CK):
                            xt = xp.tile(
                                [128, T], dt.bfloat16, tag="x", name=f"x_{s}_{k}"
                            )
                            nc.sync.dma_start(
                                out=xt, in_=x_d.ap()[s, k * 128 : (k + 1) * 128, :]
                            )
                            xts[(s, k)] = xt

                    # mm1 + tanh for both samples of the pair
                    for s in pair:
                        pse = psp.tile(
                            [128, T], dt.float32, tag="ps", name=f"pse_{s}"
                        )
                        for j in range(NJ):
                            for k in range(CK):
                                nc.tensor.matmul(
                                    pse[:, j * 512 : (j + 1) * 512],
                                    lhsT=wt_sb[:, k, :],
                                    rhs=xts[(s, k)][:, j * 512 : (j + 1) * 512],
                                    start=(k == 0),
                                    stop=(k == CK - 1),
                                )
                        e_sb = ep.tile([128, T], dt.bfloat16, tag="e", name=f"e_{s}")
                        nc.scalar.activation(
                            out=e_sb, in_=pse, func=AF.Tanh, bias=bt_sb, scale=1.0
                        )
                        esbs[s] = e_sb

                    def s2_stage(s, c, p2):
                        if (s * CK + c) % 16 == 0:
                            nc.vector.tensor_reduce(
                                out=stats[s][2][:, c : c + 1],
                                in_=p2,
                                op=OP.add,
                                axis=mybir.AxisListType.X,
                            )
                        else:
                            junk = prp.tile(
                                [128, T], dt.bfloat16, tag="junk",
                                name=f"junk_{s}_{c}",
                            )
                            nc.scalar.activation(
                                out=junk,
                                in_=p2,
                                func=AF.Copy,
                                accum_out=stats[s][2][:, c : c + 1],
                            )

                    pending = []
                    for c in range(CK):
                        for s in pair:
                            S0, S1, S2 = stats[s]
                            e_sb = esbs[s]
                            mneg_sb = mnegs[s]
                            expm = xpm.tile(
                                [128, T], dt.bfloat16, tag="expm",
                                name=f"expm_{s}_{c}",
                            )
                            pa = psp.tile(
                                [128, T], dt.float32, tag="ps", name=f"pa_{s}_{c}"
                            )
                            for jj in range(NJ):
                                nc.tensor.matmul(
                                    pa[:, jj * 512 : (jj + 1) * 512],
                                    lhsT=wa_sb[:, c * 128 : (c + 1) * 128],
                                    rhs=e_sb[:, jj * 512 : (jj + 1) * 512],
                                    start=True,
                                    stop=False,
                                )
                            for jj in range(NJ):
                                nc.tensor.matmul(
                                    pa[:, jj * 512 : (jj + 1) * 512],
                                    lhsT=ones_sb[:, :],
                                    rhs=mneg_sb[:, jj * 512 : (jj + 1) * 512],
                                    start=False,
                                    stop=True,
                                )
                            nc.scalar.activation(
                                out=expm,
                                in_=pa,
                                func=AF.Exp,
                                accum_out=S0[:, c : c + 1],
                            )
                            p1 = prp.tile(
                                [128, T], dt.bfloat16, tag="p1", name=f"p1_{s}_{c}"
                            )
                            nc.vector.tensor_tensor(
                                out=p1, in0=expm, in1=xts[(s, c)], op=OP.mult
                            )
                            p2 = prp.tile(
                                [128, T], dt.bfloat16, tag="p2", name=f"p2_{s}_{c}"
                            )
                            nc.vector.tensor_tensor(
                                out=p2, in0=p1, in1=xts[(s, c)], op=OP.mult
                            )
                            nc.vector.tensor_reduce(
                                out=S1[:, c : c + 1],
                                in_=p1,
                                op=OP.add,
                                axis=mybir.AxisListType.X,
                            )
                            pending.append((s, c, p2))
                            if len(pending) > 2:
                                s2_stage(*pending.pop(0))
                    for item in pending:
                        s2_stage(*item)

                # ---- tail: mean/std + output DMA --------------------------
                for s in range(SPC):
                    S0, S1, S2 = stats[s]
                    r0 = tailp.tile([128, CK], dt.float32, tag="r0", name=f"r0_{s}")
                    nc.vector.reciprocal(out=r0, in_=S0)
                    mean = tailp.tile(
                        [128, CK], dt.float32, tag="mean", name=f"mean_{s}"
                    )
                    nc.vector.tensor_tensor(out=mean, in0=S1, in1=r0, op=OP.mult)
                    ex2 = tailp.tile([128, CK], dt.float32, tag="ex2", name=f"ex2_{s}")
                    nc.vector.tensor_tensor(out=ex2, in0=S2, in1=r0, op=OP.mult)
                    m2 = tailp.tile([128, CK], dt.float32, tag="m2", name=f"m2_{s}")
                    nc.vector.tensor_tensor(out=m2, in0=mean, in1=mean, op=OP.mult)
                    var = tailp.tile([128, CK], dt.float32, tag="var", name=f"var_{s}")
                    nc.vector.tensor_tensor(out=var, in0=ex2, in1=m2, op=OP.subtract)
                    nc.vector.tensor_scalar(
                        out=var,
                        in0=var,
                        scalar1=1e-9,
                        scalar2=None,
                        op0=OP.max,
                    )
                    std = tailp.tile([128, CK], dt.float32, tag="std", name=f"std_{s}")
                    nc.scalar.activation(out=std, in_=var, func=AF.Sqrt)
                    nc.sync.dma_start(
                        out=out_d.ap()[s, 0:C].rearrange("(ck p) -> p ck", p=128),
                        in_=mean,
                    )
                    nc.sync.dma_start(
                        out=out_d.ap()[s, C : 2 * C].rearrange(
                            "(ck p) -> p ck", p=128
                        ),
                        in_=std,
                    )

    nc.compile()
    nc.m = get_hw_module(nc.m)
    return nc


def _get_program():
    if "nc" not in _PROG_CACHE:
        _PROG_CACHE["nc"] = _build_program()
    return _PROG_CACHE["nc"]


def _prep_inputs(x, padding_mask, W_tdnn, b_tdnn, W_attn, b_attn):
    """Host-side prep: cast/transpose, build per-core input maps."""
    xb = np.ascontiguousarray(x).astype(BF16)
    maskneg = np.where(padding_mask, np.float32(-1e9), np.float32(0.0)).astype(BF16)
    wt = np.ascontiguousarray(W_tdnn.T).astype(BF16)  # (C, BN)
    wa = np.ascontiguousarray(W_attn.T).astype(BF16)  # (BN, C)
    bt = np.ascontiguousarray(b_tdnn.astype(np.float32).reshape(BN, 1))
    in_maps = []
    for i in range(NCORES):
        sl = slice(i * SPC, (i + 1) * SPC)
        in_maps.append(
            {
                "x": np.ascontiguousarray(xb[sl]),
                "maskneg": np.ascontiguousarray(maskneg[sl]),
                "wt": wt,
                "wa": wa,
                "bt": bt,
            }
        )
    return in_maps


def kernel(x, padding_mask, W_tdnn, b_tdnn, W_attn, b_attn):
    from concourse.bass_utils import run_bass_kernel_spmd

    nc = _get_program()
    in_maps = _prep_inputs(x, padding_mask, W_tdnn, b_tdnn, W_attn, b_attn)
    res = run_bass_kernel_spmd(nc, in_maps, core_ids=list(range(NCORES)))
    out = np.concatenate([res.results[i]["out"] for i in range(NCORES)], axis=0)
    return out.astype(np.float32)



# revision 2
# speedup vs baseline: 1.0642x; 1.0642x over previous
"""AttentiveStatsPooling Trainium2 kernel (v2: mask-compacted, 3-engine).

Full-input contract: kernel(**inputs) takes the unsharded numpy inputs
  x            (32, 1536, 2048) f32
  padding_mask (32, 2048)       bool
  W_tdnn       (128, 1536)      f32
  b_tdnn       (128,)           f32
  W_attn       (1536, 128)      f32
  b_attn       (1536,)          f32
and returns the full (32, 3072) f32 output.

Key ideas over the naive schedule:
  * Masked time-steps contribute exactly 0 to every softmax sum (their
    logits are -1e9), so the host compacts each sample to its valid
    columns (~50% of T) and zero-pads to a static per-slot width.
    Samples are dealt to (core, slot) by sorted length rank so the same
    widths work for every core in SPMD. This halves ALL device work and
    removes the additive-mask matmul entirely.
  * Zero-padded columns pass x=0 through the pipeline: they contaminate
    only S0 (by k_s * exp(a0), a0 = W_attn @ tanh(b_tdnn)); a0 is
    recomputed ON DEVICE through the same mm/activation-table path so
    the correction subtracts the exact values the accumulator added.
  * S0 rides the ACT accumulator of the exp op (free); S1 uses the
    fused DVE scalar_tensor_tensor (product p1 + fp32 row-sum in one
    1x-rate op, cheaper than 2x product + 1x reduce); S2 work is split
    across DVE / ACT(copy+accum) / Pool(gpsimd tensor_tensor) by a
    greedy balance over HW-measured per-op costs.
  * bf16 everywhere with fp32 accumulation (PSUM / engine accumulators).

Measured per-op costs used for balancing (ns, [128,W] tiles):
  dve tt 0.59*W | dve stt 1.15*W | dve reduce 0.90*W
  act exp+acc .83*W+460 | act copy+acc .83*W+505 | pool tt 2.08*W
"""

import numpy as np
import ml_dtypes

B, C, T = 32, 1536, 2048
BN = 128
NCORES = 8
SPC = B // NCORES  # sample slots per core
CK = C // 128      # channel chunks of 128 partitions

BF16 = ml_dtypes.bfloat16

_PROG_CACHE = {}
_CUR_WIDTHS = None  # tuple of SPC slot widths, set by _prep_inputs
_CUR_PERM = None    # sample permutation, set by _prep_inputs


def _groups512(w):
    """Bank-aligned column groups covering [0, w)."""
    return [(j, min(j + 512, w)) for j in range(0, w, 512)]


def _schedule(widths):
    """Greedy S2-path assignment per (slot, chunk) balancing engine loads.

    Paths: 'a' = DVE stt(p1,x)        (DVE 1.15W)
           'd' = DVE tt p2 + ACT red  (DVE .59W, ACT .83W+505)
           'g' = Pool tt p2 + DVE red (Pool 2.08W, DVE .90W)
           'b' = Pool tt p2 + ACT red (Pool 2.08W, ACT .83W+505)
    """
    stt = lambda w: 1.15 * w + 80
    tt = lambda w: 0.59 * w + 80
    red = lambda w: 0.90 * w + 80
    act_red = lambda w: 0.83 * w + 505
    pool_tt = lambda w: 2.08 * w + 150
    act_exp = lambda w: 0.83 * w + 460

    # bases
    act = sum(act_exp(w) for w in widths) * CK
    act += sum(len(_groups512(w)) * 700 for w in widths)  # mm1 tanh
    act += 4 * 300  # tail sqrts
    dve = sum(stt(w) for w in widths) * CK  # S1 always fused stt
    dve += 4 * 8 * 150  # tail small ops
    pool = 0.0

    plan = {}
    for c in range(CK):
        for s in range(SPC):
            w = widths[s]
            cands = {
                "a": (dve + stt(w), act, pool),
                "d": (dve + tt(w), act + act_red(w), pool),
                "g": (dve + red(w), act, pool + pool_tt(w)),
                "b": (dve, act + act_red(w), pool + pool_tt(w)),
            }
            best = min(cands, key=lambda k: (max(cands[k]), sum(cands[k])))
            dve, act, pool = cands[best]
            plan[(s, c)] = best
    return plan


def _build_program(reps=None, widths=None):
    """Build the per-core program. reps=None: straight-line body.
    reps=K: wrap the body in a hardware For_i loop (timing only)."""
    import concourse.bacc as bacc
    import concourse.tile as tile
    import concourse.mybir as mybir
    from contextlib import nullcontext
    from concourse.bass_interp import get_hw_module

    if widths is None:
        widths = _CUR_WIDTHS
    widths = tuple(int(w) for w in widths)
    wmax = max(widths)
    plan = _schedule(widths)

    dt = mybir.dt
    AF = mybir.ActivationFunctionType
    OP = mybir.AluOpType
    AX = mybir.AxisListType

    nc = bacc.Bacc(
        "TRN2",
        target_bir_lowering=False,
        debug=False,
        num_devices=NCORES,
        num_swdge_queues=4,
    )
    x_d = [
        nc.dram_tensor(f"x{s}", [C, widths[s]], dt.bfloat16, kind="ExternalInput")
        for s in range(SPC)
    ]
    negks_d = nc.dram_tensor("negks", [SPC, 128, 1], dt.float32,
                             kind="ExternalInput")
    wt_d = nc.dram_tensor("wt", [C, BN], dt.bfloat16, kind="ExternalInput")
    wa_d = nc.dram_tensor("wa", [BN, C], dt.bfloat16, kind="ExternalInput")
    bt_d = nc.dram_tensor("bt", [BN, 1], dt.float32, kind="ExternalInput")
    out_d = nc.dram_tensor("out", [SPC, 2 * C], dt.float32, kind="ExternalOutput")

    with tile.TileContext(nc) as tc:
        with (
            tc.tile_pool(name="const", bufs=1) as constp,
            tc.tile_pool(name="xin", bufs=1) as xp,
            tc.tile_pool(name="esb", bufs=1) as ep,
            tc.tile_pool(name="expm", bufs=3) as xpm,
            tc.tile_pool(name="prod", bufs=3) as prp,
            tc.tile_pool(name="prod2", bufs=3) as pr2p,
            tc.tile_pool(name="stats", bufs=1) as statsp,
            tc.tile_pool(name="tail", bufs=2) as tailp,
            tc.tile_pool(name="pse", bufs=2, space="PSUM") as psep,
            tc.tile_pool(name="pa", bufs=2, space="PSUM") as psp,
        ):
            # ---- constants ------------------------------------------------
            wt_sb = constp.tile([128, CK, BN], dt.bfloat16, tag="wt")
            nc.sync.dma_start(
                out=wt_sb, in_=wt_d.ap().rearrange("(k p) o -> p k o", p=128)
            )
            wa_sb = constp.tile([128, C], dt.bfloat16, tag="wa")
            nc.sync.dma_start(out=wa_sb, in_=wa_d.ap())
            bt_sb = constp.tile([128, 1], dt.float32, tag="bt")
            nc.sync.dma_start(out=bt_sb, in_=bt_d.ap())
            negks_sb = constp.tile([128, SPC], dt.float32, tag="negks")
            nc.sync.dma_start(
                out=negks_sb, in_=negks_d.ap().rearrange("s p o -> p (s o)")
            )
            dummyv = constp.tile([128, 1], dt.bfloat16, tag="dumv")
            nc.vector.memset(dummyv, 0.0)

            # pad-column S0 correction base: expa0 = exp(W_attn @ tanh(b))
            # computed through the SAME engine paths as the contamination.
            e0 = constp.tile([128, 1], dt.bfloat16, tag="e0")
            nc.scalar.activation(out=e0, in_=bt_sb, func=AF.Tanh, bias=bt_sb,
                                 scale=0.0)
            pa0 = psep.tile([128, 512], dt.float32, tag="pse", name="pa0")
            for c in range(CK):
                nc.tensor.matmul(
                    pa0[:, c : c + 1],
                    lhsT=wa_sb[:, c * 128 : (c + 1) * 128],
                    rhs=e0,
                    start=True,
                    stop=True,
                )
            expa0 = constp.tile([128, CK], dt.float32, tag="expa0")
            nc.scalar.activation(out=expa0, in_=pa0[:, 0:CK], func=AF.Exp)

            loop_cm = tc.For_i(0, reps, 1) if reps is not None else nullcontext()
            with loop_cm:
                stats = []
                for s in range(SPC):
                    S0 = statsp.tile([128, CK], dt.float32, tag=f"S0_{s}")
                    S1 = statsp.tile([128, CK], dt.float32, tag=f"S1_{s}")
                    S2 = statsp.tile([128, CK], dt.float32, tag=f"S2_{s}")
                    stats.append((S0, S1, S2))

                for pair in ((0, 1), (2, 3)):
                    xts = {}
                    esbs = {}
                    for s in pair:
                        for k in range(CK):
                            xt = xp.tile(
                                [128, widths[s]], dt.bfloat16, tag=f"x{s}",
                                bufs=CK, name=f"x_{s}_{k}",
                            )
                            nc.sync.dma_start(
                                out=xt, in_=x_d[s].ap()[k * 128 : (k + 1) * 128, :]
                            )
                            xts[(s, k)] = xt

                    # mm1 + tanh -> e (bf16, [128, W_s])
                    for s in pair:
                        w = widths[s]
                        e_sb = ep.tile([128, w], dt.bfloat16, tag=f"e{s}",
                                       name=f"e_{s}")
                        for j, (j0, j1) in enumerate(_groups512(w)):
                            pse = psep.tile([128, 512], dt.float32, tag="pse",
                                            name=f"pse_{s}_{j}")
                            for k in range(CK):
                                nc.tensor.matmul(
                                    pse[:, 0 : j1 - j0],
                                    lhsT=wt_sb[:, k, :],
                                    rhs=xts[(s, k)][:, j0:j1],
                                    start=(k == 0),
                                    stop=(k == CK - 1),
                                )
                            nc.scalar.activation(
                                out=e_sb[:, j0:j1], in_=pse[:, 0 : j1 - j0],
                                func=AF.Tanh, bias=bt_sb, scale=1.0,
                            )
                        esbs[s] = e_sb

                    # chunk streams, pair-interleaved
                    pending = []
                    for c in range(CK):
                        for s in pair:
                            w = widths[s]
                            S0, S1, S2 = stats[s]
                            pa = psp.tile([128, 1536], dt.float32, tag="pa",
                                          name=f"pa_{s}_{c}")
                            for (j0, j1) in _groups512(w):
                                nc.tensor.matmul(
                                    pa[:, j0:j1],
                                    lhsT=wa_sb[:, c * 128 : (c + 1) * 128],
                                    rhs=esbs[s][:, j0:j1],
                                    start=True,
                                    stop=True,
                                )
                            expm = xpm.tile([128, wmax], dt.bfloat16, tag="expm",
                                            name=f"expm_{s}_{c}")
                            nc.scalar.activation(
                                out=expm[:, 0:w], in_=pa[:, 0:w], func=AF.Exp,
                                accum_out=S0[:, c : c + 1],
                            )
                            p1 = prp.tile([128, wmax], dt.bfloat16, tag="p1",
                                          name=f"p1_{s}_{c}")
                            nc.vector.scalar_tensor_tensor(
                                out=p1[:, 0:w], in0=expm[:, 0:w], scalar=1.0,
                                in1=xts[(s, c)], op0=OP.mult, op1=OP.mult,
                                accum_out=S1[:, c : c + 1],
                            )
                            mode = plan[(s, c)]
                            if mode == "a":
                                nc.vector.scalar_tensor_tensor(
                                    out=dummyv.broadcast_to((128, w)),
                                    in0=p1[:, 0:w], scalar=1.0,
                                    in1=xts[(s, c)], op0=OP.mult, op1=OP.mult,
                                    accum_out=S2[:, c : c + 1],
                                )
                            else:
                                p2 = pr2p.tile([128, wmax], dt.bfloat16,
                                               tag="p2", name=f"p2_{s}_{c}")
                                eng = nc.vector if mode == "d" else nc.gpsimd
                                eng.tensor_tensor(
                                    out=p2[:, 0:w], in0=p1[:, 0:w],
                                    in1=xts[(s, c)], op=OP.mult,
                                )
                                pending.append((mode, s, c, w, p2))
                            while len(pending) > 2:
                                _drain(nc, stats, pending.pop(0), AF, OP, AX,
                                       dummyv)
                    for item in pending:
                        _drain(nc, stats, item, AF, OP, AX, dummyv)

                # ---- tail: S0 correction, mean/std, output DMA ------------
                for s in range(SPC):
                    S0, S1, S2 = stats[s]
                    S0c = tailp.tile([128, CK], dt.float32, tag="s0c",
                                     name=f"s0c_{s}")
                    nc.vector.scalar_tensor_tensor(
                        out=S0c, in0=expa0, scalar=negks_sb[:, s : s + 1],
                        in1=S0, op0=OP.mult, op1=OP.add,
                    )
                    r0 = tailp.tile([128, CK], dt.float32, tag="r0",
                                    name=f"r0_{s}")
                    nc.vector.reciprocal(out=r0, in_=S0c)
                    mean = tailp.tile([128, CK], dt.float32, tag="mean",
                                      name=f"mean_{s}")
                    nc.vector.tensor_tensor(out=mean, in0=S1, in1=r0,
                                            op=OP.mult)
                    ex2 = tailp.tile([128, CK], dt.float32, tag="ex2",
                                     name=f"ex2_{s}")
                    nc.vector.tensor_tensor(out=ex2, in0=S2, in1=r0,
                                            op=OP.mult)
                    m2 = tailp.tile([128, CK], dt.float32, tag="m2",
                                    name=f"m2_{s}")
                    nc.vector.tensor_tensor(out=m2, in0=mean, in1=mean,
                                            op=OP.mult)
                    var = tailp.tile([128, CK], dt.float32, tag="var",
                                     name=f"var_{s}")
                    nc.vector.tensor_tensor(out=var, in0=ex2, in1=m2,
                                            op=OP.subtract)
                    nc.vector.tensor_scalar(
                        out=var, in0=var, scalar1=1e-9, scalar2=None,
                        op0=OP.max,
                    )
                    std = tailp.tile([128, CK], dt.float32, tag="std",
                                     name=f"std_{s}")
                    nc.scalar.activation(out=std, in_=var, func=AF.Sqrt)
                    nc.sync.dma_start(
                        out=out_d.ap()[s, 0:C].rearrange("(ck p) -> p ck", p=128),
                        in_=mean,
                    )
                    nc.sync.dma_start(
                        out=out_d.ap()[s, C : 2 * C].rearrange(
                            "(ck p) -> p ck", p=128
                        ),
                        in_=std,
                    )

    nc.compile()
    nc.m = get_hw_module(nc.m)
    return nc


def _drain(nc, stats, item, AF, OP, AX, dummyv):
    """Emit the delayed reduce stage of an S2 path."""
    mode, s, c, w, p2 = item
    S2 = stats[s][2]
    if mode == "g":
        nc.vector.tensor_reduce(
            out=S2[:, c : c + 1], in_=p2[:, 0:w], op=OP.add, axis=AX.X
        )
    else:  # 'd' or 'b': ACT copy + accumulator
        nc.scalar.activation(
            out=dummyv.broadcast_to((128, w)), in_=p2[:, 0:w], func=AF.Copy,
            accum_out=S2[:, c : c + 1],
        )


def _get_program(widths):
    key = tuple(widths)
    if key not in _PROG_CACHE:
        _PROG_CACHE[key] = _build_program(widths=key)
    return _PROG_CACHE[key]


def _prep_inputs(x, padding_mask, W_tdnn, b_tdnn, W_attn, b_attn):
    """Host-side prep: compact valid columns, deal samples to (core, slot)
    by length rank, build per-core input maps."""
    global _CUR_WIDTHS, _CUR_PERM

    counts = (~padding_mask).sum(1).astype(np.int64)
    order = np.argsort(-counts, kind="stable")  # ranks, longest first
    widths = []
    for j in range(SPC):
        grp_max = int(counts[order[j * NCORES : (j + 1) * NCORES]].max())
        widths.append(max(512, int(np.ceil(grp_max / 32) * 32)))
    _CUR_WIDTHS = tuple(widths)
    _CUR_PERM = order

    xb = x.astype(BF16)
    wt = np.ascontiguousarray(W_tdnn.T).astype(BF16)  # (C, BN)
    wa = np.ascontiguousarray(W_attn.T).astype(BF16)  # (BN, C)
    bt = np.ascontiguousarray(b_tdnn.astype(np.float32).reshape(BN, 1))

    in_maps = []
    for i in range(NCORES):
        m = {"wt": wt, "wa": wa, "bt": bt}
        negks = np.zeros((SPC, 128, 1), np.float32)
        for s in range(SPC):
            b = order[s * NCORES + i]
            n = int(counts[b])
            w = widths[s]
            xc = np.zeros((C, w), dtype=BF16)
            xc[:, :n] = xb[b][:, ~padding_mask[b]]
            m[f"x{s}"] = xc
            negks[s, :, 0] = -(w - n)
        m["negks"] = negks
        in_maps.append(m)
    return in_maps


def kernel(x, padding_mask, W_tdnn, b_tdnn, W_attn, b_attn):
    from concourse.bass_utils import run_bass_kernel_spmd

    in_maps = _prep_inputs(x, padding_mask, W_tdnn, b_tdnn, W_attn, b_attn)
    nc = _get_program(_CUR_WIDTHS)
    res = run_bass_kernel_spmd(nc, in_maps, core_ids=list(range(NCORES)))
    out = np.empty((B, 2 * C), np.float32)
    for i in range(NCORES):
        for s in range(SPC):
            out[_CUR_PERM[s * NCORES + i]] = res.results[i]["out"][s]
    return out


# revision 13
# speedup vs baseline: 1.6477x; 1.5483x over previous
"""AttentiveStatsPooling Trainium2 kernel (v3: mask-compacted, balanced).

Full-input contract: kernel(**inputs) takes the unsharded numpy inputs
  x            (32, 1536, 2048) f32
  padding_mask (32, 2048)       bool
  W_tdnn       (128, 1536)      f32
  b_tdnn       (128,)           f32
  W_attn       (1536, 128)      f32
  b_attn       (1536,)          f32
and returns the full (32, 3072) f32 output.

Key ideas over the naive schedule:
  * Masked time-steps contribute exactly 0 to every softmax sum (their
    logits are -1e9), so the host compacts each sample to its valid
    columns (~50% of T) and zero-pads to a static per-slot width.
    Samples are dealt to (core, slot) by sorted length rank so the same
    widths work for every core in SPMD. This halves ALL device work and
    removes the additive-mask matmul entirely.
  * Zero-padded columns pass x=0 through the pipeline: they contaminate
    only S0 (by k_s * exp(a0), a0 = W_attn @ tanh(b_tdnn)); a0 is
    recomputed ON DEVICE through the same mm/activation-table path so
    the correction subtracts the exact values the accumulator added.
  * S0 rides the ACT accumulator of the exp op (free). S1 uses the
    fused DVE scalar_tensor_tensor (product p1 + fp32 row-sum in one
    1x-rate op). S2 is balanced between DVE stt and [DVE 2x product +
    ACT Copy+accumulator] by measured cost. GPSIMD deliberately unused:
    on HW it contends with DVE (shared SBUF ports) and loses.
  * x arrives in 2 batched DMAs per sample (6 chunks each) - SP issue
    time and DMA-queue occupancy, not descriptor count, dominate.
  * Pair-1's mm1/tanh is injected into pair-0's chunk stream and tails
    run per-pair to keep ACT/DVE busy across phase transitions. The
    timing loop unrolls UNROLL bodies per For_i iteration to amortize
    the all-engine barrier.
"""

import numpy as np
import ml_dtypes

B, C, T = 32, 1536, 2048
BN = 128
NCORES = 8
SPC = B // NCORES  # sample slots per core
CK = C // 128      # channel chunks of 128 partitions
KH = CK // 2       # chunks per DMA half
UNROLL = 4

BF16 = ml_dtypes.bfloat16

_PROG_CACHE = {}
_CUR_WIDTHS = None  # tuple of SPC slot widths, set by _prep_inputs
_CUR_PERM = None    # sample permutation, set by _prep_inputs


def _groups512(w):
    """Bank-aligned column groups covering [0, w)."""
    return [(j, min(j + 512, w)) for j in range(0, w, 512)]


def _schedule(widths):
    """Greedy S2-path choice per (slot, chunk): 'a' = DVE stt,
    'd' = DVE tt + ACT Copy+accum. Balances measured engine loads."""
    stt = lambda w: 1.15 * w + 80
    tt = lambda w: 0.59 * w + 80
    act_red = lambda w: 0.83 * w + 505
    act_exp = lambda w: 0.83 * w + 460

    act = sum(act_exp(w) for w in widths) * CK
    act += sum(len(_groups512(w)) * 700 for w in widths)  # mm1 tanh
    act += 4 * 300  # tail sqrts
    dve = sum(stt(w) for w in widths) * CK  # S1 fused stt
    dve += 4 * 8 * 150  # tail small ops

    plan = {}
    for c in range(CK):
        for s in range(SPC):
            w = widths[s]
            cand_a = (dve + stt(w), act)
            cand_d = (dve + tt(w), act + act_red(w))
            if max(cand_a) <= max(cand_d):
                plan[(s, c)] = "a"
                dve, act = cand_a
            else:
                plan[(s, c)] = "d"
                dve, act = cand_d
    return plan


def _build_program(reps=None, widths=None, stage="full", plan_override=None,
                   unroll=None):
    """Build the per-core program. reps=None: straight-line single body.
    reps=K: hardware For_i loop running the body K times total (timing)."""
    import concourse.bacc as bacc
    import concourse.tile as tile
    import concourse.mybir as mybir
    from contextlib import nullcontext
    from concourse.bass_interp import get_hw_module

    if widths is None:
        widths = _CUR_WIDTHS
    widths = tuple(int(w) for w in widths)
    wmax = max(widths)
    if plan_override is None:
        plan = _schedule(widths)
    elif plan_override == "mixab":
        plan = {(s, c): ("b" if s == c % SPC else "a")
                for s in range(SPC) for c in range(CK)}
    else:
        plan = {(s, c): plan_override for s in range(SPC) for c in range(CK)}

    if unroll is None:
        unroll = UNROLL
    if reps is not None:
        while reps % unroll:
            unroll -= 1
        n_bodies, n_iters = unroll, reps // unroll
    else:
        n_bodies, n_iters = 1, None

    dt = mybir.dt
    AF = mybir.ActivationFunctionType
    OP = mybir.AluOpType
    AX = mybir.AxisListType

    nc = bacc.Bacc(
        "TRN2",
        target_bir_lowering=False,
        debug=False,
        num_devices=NCORES,
        num_swdge_queues=4,
    )
    x_d = [
        nc.dram_tensor(f"x{s}", [C, widths[s]], dt.bfloat16, kind="ExternalInput")
        for s in range(SPC)
    ]
    negks_d = nc.dram_tensor("negks", [128, SPC * CK], dt.float32,
                             kind="ExternalInput")
    wt_d = nc.dram_tensor("wt", [C, BN], dt.bfloat16, kind="ExternalInput")
    wa_d = nc.dram_tensor("wa", [BN, C], dt.bfloat16, kind="ExternalInput")
    bt_d = nc.dram_tensor("bt", [BN, 1], dt.float32, kind="ExternalInput")
    out_d = nc.dram_tensor("out", [SPC, 2 * C], dt.float32, kind="ExternalOutput")

    with tile.TileContext(nc) as tc:
        with (
            tc.tile_pool(name="const", bufs=1) as constp,
            tc.tile_pool(name="xin", bufs=2) as xp,
            tc.tile_pool(name="esb", bufs=1) as ep,
            tc.tile_pool(name="expm", bufs=6) as xpm,
            tc.tile_pool(name="prod", bufs=6) as prp,
            tc.tile_pool(name="prod2", bufs=4) as pr2p,
            tc.tile_pool(name="stats", bufs=1) as statsp,
            tc.tile_pool(name="tail", bufs=2) as tailp,
            tc.tile_pool(name="pse", bufs=2, space="PSUM") as psep,
            tc.tile_pool(name="pa", bufs=2, space="PSUM") as psp,
        ):
            # ---- constants ------------------------------------------------
            wt_sb = constp.tile([128, CK, BN], dt.bfloat16, tag="wt")
            nc.sync.dma_start(
                out=wt_sb, in_=wt_d.ap().rearrange("(k p) o -> p k o", p=128)
            )
            wa_sb = constp.tile([128, C], dt.bfloat16, tag="wa")
            nc.sync.dma_start(out=wa_sb, in_=wa_d.ap())
            bt_sb = constp.tile([128, 1], dt.float32, tag="bt")
            nc.sync.dma_start(out=bt_sb, in_=bt_d.ap())
            negks_sb = constp.tile([128, SPC * CK], dt.float32, tag="negks")
            nc.sync.dma_start(out=negks_sb, in_=negks_d.ap())
            dummyv = constp.tile([128, 1], dt.bfloat16, tag="dumv")
            nc.vector.memset(dummyv, 0.0)
            dummya = constp.tile([128, 1], dt.bfloat16, tag="duma")
            nc.vector.memset(dummya, 0.0)

            # pad-column S0 correction base: expa0 = exp(W_attn @ tanh(b))
            e0 = constp.tile([128, 1], dt.bfloat16, tag="e0")
            nc.scalar.activation(out=e0, in_=bt_sb, func=AF.Tanh, bias=bt_sb,
                                 scale=0.0)
            pa0 = psep.tile([128, 512], dt.float32, tag="pse", name="pa0")
            for c in range(CK):
                nc.tensor.matmul(
                    pa0[:, c : c + 1],
                    lhsT=wa_sb[:, c * 128 : (c + 1) * 128],
                    rhs=e0,
                    start=True,
                    stop=True,
                )
            expa0 = constp.tile([128, SPC * CK], dt.float32, tag="expa0")
            for s in range(SPC):
                nc.scalar.activation(out=expa0[:, s * CK : (s + 1) * CK],
                                     in_=pa0[:, 0:CK], func=AF.Exp)
            corr = constp.tile([128, SPC * CK], dt.float32, tag="corr")
            nc.vector.tensor_tensor(out=corr, in0=negks_sb, in1=expa0,
                                    op=OP.mult)

            def emit_dma(s, body):
                halves = []
                for h in range(2):
                    xt = xp.tile([128, KH, widths[s]], dt.bfloat16,
                                 tag=f"x{s}", name=f"x_{body}_{s}_{h}")
                    nc.sync.dma_start(
                        out=xt,
                        in_=x_d[s].ap()[h * KH * 128 : (h + 1) * KH * 128, :]
                        .rearrange("(k p) w -> p k w", p=128),
                    )
                    halves.append(xt)
                return halves

            def xchunk(xts, s, c):
                return xts[s][c // KH][:, c % KH, :]

            def emit_mm1(xts, s, body):
                w = widths[s]
                e_sb = ep.tile([128, w], dt.bfloat16, tag=f"e{s}",
                               name=f"e_{body}_{s}")
                for j, (j0, j1) in enumerate(_groups512(w)):
                    pse = psep.tile([128, 512], dt.float32, tag="pse",
                                    name=f"pse_{body}_{s}_{j}")
                    for k in range(CK):
                        nc.tensor.matmul(
                            pse[:, 0 : j1 - j0],
                            lhsT=wt_sb[:, k, :],
                            rhs=xchunk(xts, s, k)[:, j0:j1],
                            start=(k == 0),
                            stop=(k == CK - 1),
                        )
                    nc.scalar.activation(
                        out=e_sb[:, j0:j1], in_=pse[:, 0 : j1 - j0],
                        func=AF.Tanh, bias=bt_sb, scale=1.0,
                    )
                return e_sb

            def drain(item):
                s, c, w, p2 = item
                S2 = stats[s][2]
                nc.scalar.activation(
                    out=dummya.broadcast_to((128, w)), in_=p2[:, 0:w],
                    func=AF.Copy, accum_out=S2[:, c : c + 1],
                )

            def emit_chunk(xts, esbs, s, c, body, pending):
                w = widths[s]
                S0, S1, S2 = stats[s]
                pa = psp.tile([128, 1536], dt.float32, tag="pa",
                              name=f"pa_{body}_{s}_{c}")
                for (j0, j1) in _groups512(w):
                    nc.tensor.matmul(
                        pa[:, j0:j1],
                        lhsT=wa_sb[:, c * 128 : (c + 1) * 128],
                        rhs=esbs[s][:, j0:j1],
                        start=True,
                        stop=True,
                    )
                if stage == "mm2":
                    return
                expm = xpm.tile([128, wmax], dt.bfloat16, tag="expm",
                                name=f"expm_{body}_{s}_{c}")
                nc.scalar.activation(
                    out=expm[:, 0:w], in_=pa[:, 0:w], func=AF.Exp,
                    accum_out=S0[:, c : c + 1],
                )
                if stage == "exp":
                    return
                p1 = prp.tile([128, wmax], dt.bfloat16, tag="p1",
                              name=f"p1_{body}_{s}_{c}")
                if stage == "s1nacc":
                    # product only, no DVE accumulator
                    nc.vector.tensor_tensor(
                        out=p1[:, 0:w], in0=expm[:, 0:w],
                        in1=xchunk(xts, s, c), op=OP.mult,
                    )
                    return
                if stage == "s1const":
                    # stt with accum but independent of ACT output
                    nc.vector.scalar_tensor_tensor(
                        out=p1[:, 0:w], in0=xchunk(xts, s, c), scalar=1.0,
                        in1=xchunk(xts, s, c), op0=OP.mult, op1=OP.mult,
                        accum_out=S1[:, c : c + 1],
                    )
                    return
                nc.vector.scalar_tensor_tensor(
                    out=p1[:, 0:w], in0=expm[:, 0:w], scalar=1.0,
                    in1=xchunk(xts, s, c), op0=OP.mult, op1=OP.mult,
                    accum_out=S1[:, c : c + 1],
                )
                if stage == "s1":
                    return
                if plan[(s, c)] == "a":
                    p2 = pr2p.tile([128, wmax], dt.bfloat16, tag="p2",
                                   name=f"p2_{body}_{s}_{c}")
                    nc.vector.scalar_tensor_tensor(
                        out=p2[:, 0:w], in0=p1[:, 0:w],
                        scalar=1.0, in1=xchunk(xts, s, c), op0=OP.mult,
                        op1=OP.mult, accum_out=S2[:, c : c + 1],
                    )
                elif plan[(s, c)] == "adum":
                    nc.vector.scalar_tensor_tensor(
                        out=dummyv.broadcast_to((128, w)), in0=p1[:, 0:w],
                        scalar=1.0, in1=xchunk(xts, s, c), op0=OP.mult,
                        op1=OP.mult, accum_out=S2[:, c : c + 1],
                    )
                else:
                    p2 = pr2p.tile([128, wmax], dt.bfloat16, tag="p2",
                                   name=f"p2_{body}_{s}_{c}")
                    eng = nc.gpsimd if plan[(s, c)] == "b" else nc.vector
                    eng.tensor_tensor(
                        out=p2[:, 0:w], in0=p1[:, 0:w],
                        in1=xchunk(xts, s, c), op=OP.mult,
                    )
                    pending.append((s, c, w, p2))
                while len(pending) > 3:
                    drain(pending.pop(0))

            def emit_tail(statsw, body):
                S0w, S1w, S2w = statsw
                NW = SPC * CK
                S0c = tailp.tile([128, NW], dt.float32, tag="s0c",
                                 name=f"s0c_{body}")
                nc.vector.tensor_tensor(out=S0c, in0=S0w, in1=corr, op=OP.add)
                r0 = tailp.tile([128, NW], dt.float32, tag="r0",
                                name=f"r0_{body}")
                nc.vector.reciprocal(out=r0, in_=S0c)
                mean = tailp.tile([128, NW], dt.float32, tag="mean",
                                  name=f"mean_{body}")
                nc.vector.tensor_tensor(out=mean, in0=S1w, in1=r0, op=OP.mult)
                ex2 = tailp.tile([128, NW], dt.float32, tag="ex2",
                                 name=f"ex2_{body}")
                nc.vector.tensor_tensor(out=ex2, in0=S2w, in1=r0, op=OP.mult)
                m2 = tailp.tile([128, NW], dt.float32, tag="m2",
                                name=f"m2_{body}")
                nc.vector.tensor_tensor(out=m2, in0=mean, in1=mean, op=OP.mult)
                var = tailp.tile([128, NW], dt.float32, tag="var",
                                 name=f"var_{body}")
                nc.vector.tensor_tensor(out=var, in0=ex2, in1=m2,
                                        op=OP.subtract)
                nc.vector.tensor_scalar(
                    out=var, in0=var, scalar1=1e-9, scalar2=None, op0=OP.max
                )
                std = tailp.tile([128, NW], dt.float32, tag="std",
                                 name=f"std_{body}")
                nc.scalar.activation(out=std, in_=var, func=AF.Sqrt)
                for s in range(SPC):
                    nc.sync.dma_start(
                        out=out_d.ap()[s, 0:C].rearrange("(ck p) -> p ck",
                                                         p=128),
                        in_=mean[:, s * CK : (s + 1) * CK],
                    )
                    nc.sync.dma_start(
                        out=out_d.ap()[s, C : 2 * C].rearrange(
                            "(ck p) -> p ck", p=128),
                        in_=std[:, s * CK : (s + 1) * CK],
                    )

            loop_cm = (tc.For_i(0, n_iters, 1)
                       if n_iters is not None and n_iters > 1
                       else nullcontext())
            with loop_cm:
                for body in range(n_bodies):
                    S0w = statsp.tile([128, SPC * CK], dt.float32,
                                      tag="S0", name=f"S0_{body}")
                    S1w = statsp.tile([128, SPC * CK], dt.float32,
                                      tag="S1", name=f"S1_{body}")
                    S2w = statsp.tile([128, SPC * CK], dt.float32,
                                      tag="S2", name=f"S2_{body}")
                    stats = [
                        (S0w[:, s * CK : (s + 1) * CK],
                         S1w[:, s * CK : (s + 1) * CK],
                         S2w[:, s * CK : (s + 1) * CK])
                        for s in range(SPC)
                    ]
                    statsw = (S0w, S1w, S2w)

                    xts = {s: emit_dma(s, body) for s in range(SPC)}
                    if stage == "dma":
                        continue
                    esbs = {}
                    for s in range(SPC):
                        esbs[s] = emit_mm1(xts, s, body)
                    if stage == "mm1":
                        continue

                    pending = []
                    for c in range(CK):
                        for s in range(SPC):
                            emit_chunk(xts, esbs, s, c, body, pending)
                    for item in pending:
                        drain(item)
                    if stage == "full":
                        emit_tail(statsw, body)

    nc.compile()
    nc.m = get_hw_module(nc.m)
    return nc


def _get_program(widths):
    key = tuple(widths)
    if key not in _PROG_CACHE:
        _PROG_CACHE[key] = _build_program(widths=key)
    return _PROG_CACHE[key]


def _prep_inputs(x, padding_mask, W_tdnn, b_tdnn, W_attn, b_attn):
    """Host-side prep: compact valid columns, deal samples to (core, slot)
    by length rank, build per-core input maps."""
    global _CUR_WIDTHS, _CUR_PERM

    counts = (~padding_mask).sum(1).astype(np.int64)
    order = np.argsort(-counts, kind="stable")  # ranks, longest first
    widths = []
    for j in range(SPC):
        grp_max = int(counts[order[j * NCORES : (j + 1) * NCORES]].max())
        widths.append(max(512, int(np.ceil(grp_max / 32) * 32)))
    _CUR_WIDTHS = tuple(widths)
    _CUR_PERM = order

    xb = x.astype(BF16)
    wt = np.ascontiguousarray(W_tdnn.T).astype(BF16)  # (C, BN)
    wa = np.ascontiguousarray(W_attn.T).astype(BF16)  # (BN, C)
    bt = np.ascontiguousarray(b_tdnn.astype(np.float32).reshape(BN, 1))

    in_maps = []
    for i in range(NCORES):
        m = {"wt": wt, "wa": wa, "bt": bt}
        negks = np.zeros((128, SPC * CK), np.float32)
        for s in range(SPC):
            b = order[s * NCORES + i]
            n = int(counts[b])
            w = widths[s]
            xc = np.zeros((C, w), dtype=BF16)
            xc[:, :n] = xb[b][:, ~padding_mask[b]]
            m[f"x{s}"] = xc
            negks[:, s * CK : (s + 1) * CK] = -(w - n)
        m["negks"] = negks
        in_maps.append(m)
    return in_maps


def kernel(x, padding_mask, W_tdnn, b_tdnn, W_attn, b_attn):
    from concourse.bass_utils import run_bass_kernel_spmd

    in_maps = _prep_inputs(x, padding_mask, W_tdnn, b_tdnn, W_attn, b_attn)
    nc = _get_program(_CUR_WIDTHS)
    res = run_bass_kernel_spmd(nc, in_maps, core_ids=list(range(NCORES)))
    out = np.empty((B, 2 * C), np.float32)
    for i in range(NCORES):
        for s in range(SPC):
            out[_CUR_PERM[s * NCORES + i]] = res.results[i]["out"][s]
    return out
